# revision 32
# baseline (speedup 1.0000x reference)
"""Trainium2 Bass kernel for CompressedLinear: out = x @ (w_int8 * scale).T + bias.

Sharding (Megatron column-parallel): weight/scale/bias are split along the
output dim across 8 NeuronCores, x is replicated, per-core outputs are
concatenated on the feature axis.

Identity: x @ (w*scale).T + bias == (x @ w.T) * scale + bias, so the matmul
runs on integer codes (exact in fp16) and per-channel scale/bias are applied
on PSUM eviction.  x is fp16 (~2.5e-4 rel err; budget 2e-2).

Weight transport: w ships as uint8 codes (5.6MB vs 11.3MB fp16) so the
startup burst fits under the DMA rate while m-tile 0 computes.  On-chip
expansion uses the offset-1024 trick: for c in [0,127], fp16(1024+c) has bit
pattern 0x6400|c, so two full-rate DVE tensor_scalar ops per chunk (reading
u8 pairs as u16) build fp16 weights w' = 1024+c.  The matmul then computes
x@w'.T = x@w.T + 1024*rowsum(x); a host-planted zero code per k-tile makes
column 352 of each n2 PSUM group accumulate exactly 1024*rowsum(x), and
eviction computes (psum - col352)*scale + bias via scalar_tensor_tensor.

Schedule: steady state hits the PE issue-rate roofline (6144 matmuls,
128x128 stationary x, <=512-wide moving w, 32-deep K accumulation).  Edge
optimizations: pre-tiled DRAM layouts (contiguous per-partition DMA
packets), startup split across hardware rings (x+broadcasts on sync, w on
scalar) in first-consumption order, m-tile 0 k-chunk-outer across 6
concurrent PSUM banks, and HAM warmup matmuls so the PE clock gate is at
2.4GHz when real matmuls start.
"""

import numpy as np

import concourse.bass as bass
import concourse.mybir as mybir
import concourse.tile as tile
from concourse import bacc
from concourse.bass_utils import run_bass_kernel_spmd

B, S, IN, OUT = 4, 2048, 4096, 11008
N_CORES = 8
TOK = B * S
O_CORE = OUT // N_CORES
P = 128
OPAD = O_CORE + 2     # per-ktile row: 1376 codes + zero (ones col) + pad

M_TILE = 256          # tokens per m-tile (2 PSUM-partition subtiles)
N_TILE = 512          # output columns per PSUM bank
KGX = 4               # k-tiles per x DMA chunk
N_WARM = 56           # HAM warmup matmuls issued before real data lands
X_BUFS = 2
# w chunk plan in k-tiles: small leading chunks cut time-to-first-matmul
# during the DMA ramp, 4-ktile chunks amortize the rest.
W_PLAN = (1, 1, 1, 1, 2, 2, 4, 4, 4, 4, 4, 4)
# m-tile 0's x chunk plan (same idea; later m-tiles use uniform KGX chunks)
X0_PLAN = (1, 1, 2, 4, 4, 4, 4, 4, 4, 4)

KSUB = IN // P        # 32 k-tiles
MSUB = M_TILE // P    # 2
N_MT = TOK // M_TILE  # 32 m-tiles


def build_nc(n_warm=N_WARM, x_bufs=X_BUFS, kgx=KGX, w_plan=W_PLAN,
             x0_plan=X0_PLAN):
    nxc = KSUB // kgx
    assert sum(w_plan) == KSUB and sum(x0_plan) == KSUB
    # (col offset within a ktile row, matmul width, evict width)
    n_slices = [(0, 512, 512), (512, 512, 512), (1024, 353, 352)]
    groups = [(ms, n_idx) for ms in range(MSUB) for n_idx in range(3)]
    fp16 = mybir.dt.float16
    u16 = mybir.dt.uint16
    f32 = mybir.dt.float32
    AOT = mybir.AluOpType

    nc = bacc.Bacc(None, target_bir_lowering=False)
    # x pre-tiled: [m-tile, partition, kchunk-major free] (see _shard_inputs)
    xt = nc.declare_dram_parameter("xt", [N_MT, P, KSUB * M_TILE], fp16, False)
    # w codes, chunk-contiguous per the chunk plan (per-partition contiguous
    # DMA packets: nk*1378 bytes), one param per chunk-size class.
    n1c = sum(1 for nk in w_plan if nk == 1)
    n2c = sum(1 for nk in w_plan if nk == 2)
    n4c = sum(1 for nk in w_plan if nk == 4)
    wt1 = nc.declare_dram_parameter("wt1", [max(n1c, 1), P, OPAD],
                                    mybir.dt.uint8, False)
    wt2 = nc.declare_dram_parameter("wt2", [max(n2c, 1), P, 2 * OPAD],
                                    mybir.dt.uint8, False)
    wt4 = nc.declare_dram_parameter("wt4", [max(n4c, 1), P, 4 * OPAD],
                                    mybir.dt.uint8, False)
    scale = nc.declare_dram_parameter("scale", [O_CORE], f32, False)
    bias = nc.declare_dram_parameter("bias", [O_CORE], f32, False)
    out = nc.declare_dram_parameter("out", [TOK, O_CORE], f32, True)

    xt_re = xt.rearrange("m p (c e) -> m p c e", c=nxc)   # e = kgx*M_TILE
    out_re = out.rearrange("(m p) o -> m p o", p=P)

    with tile.TileContext(nc) as tc:
        with (
            tc.tile_pool(name="const", bufs=1) as cpool,
            tc.tile_pool(name="u8p", bufs=2) as u8pool,
            tc.tile_pool(name="xp", bufs=x_bufs) as xpool,
            tc.tile_pool(name="op", bufs=2) as opool,
            tc.tile_pool(name="ps", bufs=1, space="PSUM") as pspool,
        ):
            # --- PSUM banks: 6 accumulation groups (allocated first => bank
            # aligned), warmup bank last.
            def ps_tile(g):
                return pspool.tile([P, N_TILE], f32, tag=f"ps{g}", name=f"ps{g}")

            ps_list = [ps_tile(g) for g in range(len(groups))]

            # --- HAM warmup: PE busy from the end of the framework preamble
            # so the clock gate is at 8/8 before real matmuls start.  Sized
            # to bridge until the first w chunk is expanded; results are
            # never read.
            if n_warm:
                warm_sb = cpool.tile([P, P], fp16, tag="warm_sb")
                nc.vector.memset(warm_sb[:], 0.0)
                warm_ps = pspool.tile([P, 64], f32, tag="warm_ps")
                for _ in range(n_warm):
                    nc.tensor.matmul(warm_ps[:], lhsT=warm_sb[:],
                                     rhs=warm_sb[:, :64], start=True, stop=True)

            # --- startup DMAs, first-consumption order.
            # w uint8 chunks on the scalar ring, then expansion codes ->
            # fp16(1024+c) on the vector engine: u8 pairs read as u16, two
            # strided bitwise tensor_scalar ops per chunk.
            u8s = []
            w_srcs = {1: (wt1, 0), 2: (wt2, 0), 4: (wt4, 0)}
            for c, nk in enumerate(w_plan):
                u8 = u8pool.tile([P, nk * OPAD], mybir.dt.uint8, tag=f"u8_{nk}",
                                 name=f"u8_{c}", bufs=(2 if nk < 4 else 4))
                src, idx = w_srcs[nk]
                w_srcs[nk] = (src, idx + 1)
                nc.scalar.dma_start(out=u8[:], in_=src[idx])
                u8s.append(u8)
            w16s = []       # per chunk
            w_loc = []      # per ktile: (chunk idx, offset of ktile row)
            for c, nk in enumerate(w_plan):
                w16 = cpool.tile([P, nk * OPAD], fp16, tag=f"w{c}",
                                 name=f"w16_{c}")
                u16v = u8s[c].bitcast(u16)
                wv = w16.bitcast(u16).rearrange("p (n two) -> p n two", two=2)
                nc.vector.tensor_scalar(out=wv[:, :, 0], in0=u16v[:],
                                        scalar1=0x7F, scalar2=0x6400,
                                        op0=AOT.bitwise_and, op1=AOT.bitwise_or)
                nc.vector.tensor_scalar(out=wv[:, :, 1], in0=u16v[:],
                                        scalar1=8, scalar2=0x6400,
                                        op0=AOT.logical_shift_right,
                                        op1=AOT.bitwise_or)
                w16s.append(w16)
                w_loc.extend((c, i * OPAD) for i in range(nk))

            def x_chunk(mi, c):
                x_sb = xpool.tile([P, kgx * M_TILE], fp16, tag=f"x{c}",
                                  name=f"x{mi}_{c}")
                nc.sync.dma_start(out=x_sb[:], in_=xt_re[mi][:, c, :])
                return x_sb

            def mm(ps, x_sb, kt, ks, ms, n0, nmm):
                wc, woff = w_loc[ks]
                nc.tensor.matmul(
                    ps[:, :nmm],
                    lhsT=x_sb[:, kt * M_TILE + ms * P: kt * M_TILE + ms * P + P],
                    rhs=w16s[wc][:, woff + n0: woff + n0 + nmm],
                    start=(ks == 0), stop=(ks == KSUB - 1),
                )

            def evict(mi, out_sb, g, rs):
                # rs: the CURRENT m-tile's n2-group psum; col 352 = 1024*rowsum
                ms, n_idx = groups[g]
                n0, nmm, nev = n_slices[n_idx]
                o0 = ms * O_CORE + n0
                nc.vector.scalar_tensor_tensor(
                    out=out_sb[:, o0:o0 + nev], in0=ps_list[g][:, :nev],
                    scalar=rs[:, 352:353], in1=scale_sb[:, n0:n0 + nev],
                    op0=AOT.subtract, op1=AOT.mult)
                nc.vector.tensor_add(out=out_sb[:, o0:o0 + nev],
                                     in0=out_sb[:, o0:o0 + nev],
                                     in1=bias_sb[:, n0:n0 + nev])
                if mi < N_MT - 1:
                    ps_list[g] = ps_tile(g)  # next m-tile's tile, same bank
                nc.scalar.dma_start(out=out_re[mi * MSUB + ms][:, n0:n0 + nev],
                                    in_=out_sb[:, o0:o0 + nev])

            # --- m-tile 0: k-chunk-outer over 6 concurrent PSUM groups so
            # compute starts after the first x/w chunks land and tracks the
            # weight DMA+expansion stream.  Its x rides the finer X0 plan.
            x0_tiles = []
            k0 = 0
            for c, nk in enumerate(x0_plan):
                x_sb = xpool.tile([P, nk * M_TILE], fp16, tag=f"x0_{c}",
                                  name=f"x0_{c}", bufs=1)
                nc.sync.dma_start(
                    out=x_sb[:],
                    in_=xt[0][:, k0 * M_TILE:(k0 + nk) * M_TILE])
                x0_tiles.append(x_sb)
                k0 += nk
            # scale/bias broadcasts ride the sync ring after m-tile 0's x,
            # before the m-tile 1 prefetch (needed by first eviction ~40us).
            scale_sb = cpool.tile([P, O_CORE], f32, tag="scale_sb")
            nc.sync.dma_start(out=scale_sb[:],
                              in_=scale[None, :].to_broadcast((P, O_CORE)))
            bias_sb = cpool.tile([P, O_CORE], f32, tag="bias_sb")
            nc.sync.dma_start(out=bias_sb[:],
                              in_=bias[None, :].to_broadcast((P, O_CORE)))
            out_sb = opool.tile([P, MSUB * O_CORE], f32, tag="o", name="o0")
            k0 = 0
            for c, nk in enumerate(x0_plan):
                for g, (ms, n_idx) in enumerate(groups):
                    n0, nmm, _ = n_slices[n_idx]
                    for kt in range(nk):
                        mm(ps_list[g], x0_tiles[c], kt, k0 + kt, ms, n0, nmm)
                k0 += nk
            rs0 = [ps_list[ms * 3 + 2] for ms in range(MSUB)]
            for g, (ms, n_idx) in enumerate(groups):
                evict(0, out_sb, g, rs0[ms])

            # --- steady state: group-outer (full-K accumulation per group).
            # Within each ms-half the n2 group (whose col 352 carries the
            # rowsum term every eviction reads) runs FIRST and evicts
            # immediately (the pool keeps its bank readable until every rs
            # reader is done), so only n1's eviction trails the half's last
            # matmul (minimizes the kernel tail).
            for mi in range(1, N_MT):
                xc = [x_chunk(mi, c) for c in range(nxc)]
                out_sb = opool.tile([P, MSUB * O_CORE], f32, tag="o",
                                    name=f"o{mi}")
                for ms in range(MSUB):
                    rs = None
                    for n_idx in (2, 0, 1):
                        g = ms * 3 + n_idx
                        n0, nmm, _ = n_slices[n_idx]
                        for ks in range(KSUB):
                            mm(ps_list[g], xc[ks // kgx], ks % kgx, ks, ms,
                               n0, nmm)
                        if rs is None:
                            rs = ps_list[g]
                        evict(mi, out_sb, g, rs)
    nc.compile()
    return nc


def _shard_inputs(x2d, w, scale, bias, n_cores=N_CORES, o_core=O_CORE,
                  kgx=KGX, w_plan=W_PLAN):
    # x: [TOK, IN] f32 -> fp16, tiled [N_MT, P, KSUB*M_TILE] with free dim
    # grouped as (chunk, ktile-in-chunk, token) so each chunk is contiguous.
    xt = np.ascontiguousarray(x2d.T).astype(np.float16)       # [IN, TOK]
    xt = xt.reshape(KSUB, P, N_MT, M_TILE)                     # ks p m t
    xt = xt.transpose(2, 1, 0, 3)                              # m p ks t
    xt = np.ascontiguousarray(xt.reshape(N_MT, P, KSUB * M_TILE))
    in_maps = []
    for c in range(n_cores):
        sl = slice(c * o_core, (c + 1) * o_core)
        wtc = np.ascontiguousarray(w[sl].T).astype(np.uint8)   # [IN, o_core]
        wpad = np.zeros((KSUB, P, OPAD), dtype=np.uint8)
        wpad[:, :, :o_core] = wtc.reshape(KSUB, P, o_core)     # pad cols = 0
        by_nk = {1: [], 2: [], 4: []}
        k0 = 0
        for nk in w_plan:
            chunk = wpad[k0:k0 + nk].transpose(1, 0, 2).reshape(P, nk * OPAD)
            by_nk[nk].append(chunk)
            k0 += nk
        def stackc(lst, nk):
            if not lst:
                return np.zeros((1, P, nk * OPAD), dtype=np.uint8)
            return np.ascontiguousarray(np.stack(lst))
        in_maps.append({
            "xt": xt,
            "wt1": stackc(by_nk[1], 1),
            "wt2": stackc(by_nk[2], 2),
            "wt4": stackc(by_nk[4], 4),
            "scale": np.ascontiguousarray(scale[sl]).astype(np.float32),
            "bias": np.ascontiguousarray(bias[sl]).astype(np.float32),
        })
    return in_maps


def _ensure_ntff_hook():
    """Register the axon NTFF profiling hook if the image's antenv lacks it."""
    import sys, types
    try:
        from antenv.axon_hooks import get_axon_ntff_profile_hook  # noqa: F401
        return
    except ImportError:
        pass
    try:
        import antenv
        from trn_agent_boot.trn_boot import _ntff_profile_via_ctypes
        mod = types.ModuleType("antenv.axon_hooks")
        _hook = [_ntff_profile_via_ctypes("/opt/axon/libaxon_pjrt.so")]
        mod.set_axon_ntff_profile_hook = lambda h: _hook.__setitem__(0, h)
        mod.get_axon_ntff_profile_hook = lambda: _hook[0]
        sys.modules["antenv.axon_hooks"] = mod
        antenv.axon_hooks = mod
    except Exception as e:  # profiling is best-effort; execution still works
        print(f"NTFF hook registration failed: {e}")


def run_hw(x2d, w, scale, bias, trace=False, **build_kwargs):
    """Run sharded on 8 cores; returns (full [TOK, OUT] f32 output, exec_time_ns)."""
    if trace:
        _ensure_ntff_hook()
    nc = build_nc(**build_kwargs)
    in_maps = _shard_inputs(x2d, w, scale, bias,
                            kgx=build_kwargs.get("kgx", KGX),
                            w_plan=build_kwargs.get("w_plan", W_PLAN))
    last_err = None
    for attempt in range(3):
        try:
            res = run_bass_kernel_spmd(nc, in_maps, core_ids=list(range(N_CORES)),
                                       trace=trace)
            out = np.concatenate([res.results[c]["out"] for c in range(N_CORES)],
                                 axis=1)
            return out, res.exec_time_ns
        except Exception as e:  # transient NRT_EXEC_UNIT_UNRECOVERABLE etc.
            last_err = e
            print(f"run attempt {attempt} failed: {type(e).__name__}: {e}")
            try:
                import jax
                import jax.extend.backend as _jb
                jax.clear_caches()
                _jb.clear_backends()
            except Exception as e2:
                print(f"backend reset failed: {e2}")
            import time
            time.sleep(5)
    raise last_err


def kernel(**inputs):
    x = np.asarray(inputs["x"], dtype=np.float32)
    w = np.asarray(inputs["weight_int8"])
    scale = np.asarray(inputs["scale"], dtype=np.float32)
    bias = np.asarray(inputs["bias"], dtype=np.float32)
    out2d, _ = run_hw(x.reshape(TOK, IN), w, scale, bias, trace=False)
    return out2d.reshape(B, S, OUT)


# revision 33
# speedup vs baseline: 1.0057x; 1.0057x over previous
"""Trainium2 Bass kernel for CompressedLinear: out = x @ (w_int8 * scale).T + bias.

Sharding (Megatron column-parallel): weight/scale/bias are split along the
output dim across 8 NeuronCores, x is replicated, per-core outputs are
concatenated on the feature axis.

Identity: x @ (w*scale).T + bias == (x @ w.T) * scale + bias, so the matmul
runs on integer codes (exact in fp16) and per-channel scale/bias are applied
on PSUM eviction.  x is fp16 (~2.5e-4 rel err; budget 2e-2).

Weight transport: w ships as uint8 codes (5.6MB vs 11.3MB fp16) so the
startup burst fits under the DMA rate while m-tile 0 computes.  On-chip
expansion uses the offset-1024 trick: for c in [0,127], fp16(1024+c) has bit
pattern 0x6400|c, so two full-rate DVE tensor_scalar ops per chunk (reading
u8 pairs as u16) build fp16 weights w' = 1024+c.  The matmul then computes
x@w'.T = x@w.T + 1024*rowsum(x); a host-planted zero code per k-tile makes
column 352 of each n2 PSUM group accumulate exactly 1024*rowsum(x), and
eviction computes (psum - col352)*scale + bias via scalar_tensor_tensor.

Schedule: steady state hits the PE issue-rate roofline (6144 matmuls,
128x128 stationary x, <=512-wide moving w, 32-deep K accumulation).  Edge
optimizations: pre-tiled DRAM layouts (contiguous per-partition DMA
packets), startup split across hardware rings (x+broadcasts on sync, w on
scalar) in first-consumption order, m-tile 0 k-chunk-outer across 6
concurrent PSUM banks, and HAM warmup matmuls so the PE clock gate is at
2.4GHz when real matmuls start.
"""

import numpy as np

import concourse.bass as bass
import concourse.mybir as mybir
import concourse.tile as tile
from concourse import bacc
from concourse.bass_utils import run_bass_kernel_spmd

B, S, IN, OUT = 4, 2048, 4096, 11008
N_CORES = 8
TOK = B * S
O_CORE = OUT // N_CORES
P = 128
OPAD = O_CORE + 2     # per-ktile row: 1376 codes + zero (ones col) + pad

M_TILE = 256          # tokens per m-tile (2 PSUM-partition subtiles)
N_TILE = 512          # output columns per PSUM bank
KGX = 4               # k-tiles per x DMA chunk
N_WARM = 72           # HAM warmup matmuls issued before real data lands
X_BUFS = 2
# w chunk plan in k-tiles: small leading chunks cut time-to-first-matmul
# during the DMA ramp, 4-ktile chunks amortize the rest.
W_PLAN = (1, 1, 2, 4, 4, 4, 4, 4, 4, 4)
# m-tile 0's x chunk plan (same idea; later m-tiles use uniform KGX chunks)
X0_PLAN = (4, 4, 4, 4, 4, 4, 4, 4)

KSUB = IN // P        # 32 k-tiles
MSUB = M_TILE // P    # 2
N_MT = TOK // M_TILE  # 32 m-tiles


def build_nc(n_warm=N_WARM, x_bufs=X_BUFS, kgx=KGX, w_plan=W_PLAN,
             x0_plan=X0_PLAN):
    nxc = KSUB // kgx
    assert sum(w_plan) == KSUB and sum(x0_plan) == KSUB
    # (col offset within a ktile row, matmul width, evict width)
    n_slices = [(0, 512, 512), (512, 512, 512), (1024, 353, 352)]
    groups = [(ms, n_idx) for ms in range(MSUB) for n_idx in range(3)]
    fp16 = mybir.dt.float16
    u16 = mybir.dt.uint16
    f32 = mybir.dt.float32
    AOT = mybir.AluOpType

    nc = bacc.Bacc(None, target_bir_lowering=False)
    # x pre-tiled: [m-tile, partition, kchunk-major free] (see _shard_inputs)
    xt = nc.declare_dram_parameter("xt", [N_MT, P, KSUB * M_TILE], fp16, False)
    # w codes, chunk-contiguous per the chunk plan (per-partition contiguous
    # DMA packets: nk*1378 bytes), one param per chunk-size class.
    n1c = sum(1 for nk in w_plan if nk == 1)
    n2c = sum(1 for nk in w_plan if nk == 2)
    n4c = sum(1 for nk in w_plan if nk == 4)
    wt1 = nc.declare_dram_parameter("wt1", [max(n1c, 1), P, OPAD],
                                    mybir.dt.uint8, False)
    wt2 = nc.declare_dram_parameter("wt2", [max(n2c, 1), P, 2 * OPAD],
                                    mybir.dt.uint8, False)
    wt4 = nc.declare_dram_parameter("wt4", [max(n4c, 1), P, 4 * OPAD],
                                    mybir.dt.uint8, False)
    scale = nc.declare_dram_parameter("scale", [O_CORE], f32, False)
    bias = nc.declare_dram_parameter("bias", [O_CORE], f32, False)
    out = nc.declare_dram_parameter("out", [TOK, O_CORE], f32, True)

    xt_re = xt.rearrange("m p (c e) -> m p c e", c=nxc)   # e = kgx*M_TILE
    out_re = out.rearrange("(m p) o -> m p o", p=P)

    with tile.TileContext(nc) as tc:
        with (
            tc.tile_pool(name="const", bufs=1) as cpool,
            tc.tile_pool(name="u8p", bufs=2) as u8pool,
            tc.tile_pool(name="xp", bufs=x_bufs) as xpool,
            tc.tile_pool(name="op", bufs=2) as opool,
            tc.tile_pool(name="ps", bufs=1, space="PSUM") as pspool,
        ):
            # --- PSUM banks: 6 accumulation groups (allocated first => bank
            # aligned), warmup bank last.
            def ps_tile(g):
                return pspool.tile([P, N_TILE], f32, tag=f"ps{g}", name=f"ps{g}")

            ps_list = [ps_tile(g) for g in range(len(groups))]

            # --- HAM warmup: PE busy from the end of the framework preamble
            # so the clock gate is at 8/8 before real matmuls start.  Sized
            # to bridge until the first w chunk is expanded; results are
            # never read.
            if n_warm:
                warm_sb = cpool.tile([P, P], fp16, tag="warm_sb")
                nc.vector.memset(warm_sb[:], 0.0)
                warm_ps = pspool.tile([P, 64], f32, tag="warm_ps")
                for _ in range(n_warm):
                    nc.tensor.matmul(warm_ps[:], lhsT=warm_sb[:],
                                     rhs=warm_sb[:, :64], start=True, stop=True)

            # --- startup DMAs, first-consumption order.
            # w uint8 chunks on the scalar ring, then expansion codes ->
            # fp16(1024+c) on the vector engine: u8 pairs read as u16, two
            # strided bitwise tensor_scalar ops per chunk.
            u8s = []
            w_srcs = {1: (wt1, 0), 2: (wt2, 0), 4: (wt4, 0)}
            for c, nk in enumerate(w_plan):
                u8 = u8pool.tile([P, nk * OPAD], mybir.dt.uint8, tag=f"u8_{nk}",
                                 name=f"u8_{c}", bufs=(2 if nk < 4 else 4))
                src, idx = w_srcs[nk]
                w_srcs[nk] = (src, idx + 1)
                nc.scalar.dma_start(out=u8[:], in_=src[idx])
                u8s.append(u8)
            w16s = []       # per chunk
            w_loc = []      # per ktile: (chunk idx, offset of ktile row)
            for c, nk in enumerate(w_plan):
                w16 = cpool.tile([P, nk * OPAD], fp16, tag=f"w{c}",
                                 name=f"w16_{c}")
                u16v = u8s[c].bitcast(u16)
                wv = w16.bitcast(u16).rearrange("p (n two) -> p n two", two=2)
                nc.vector.tensor_scalar(out=wv[:, :, 0], in0=u16v[:],
                                        scalar1=0x7F, scalar2=0x6400,
                                        op0=AOT.bitwise_and, op1=AOT.bitwise_or)
                nc.vector.tensor_scalar(out=wv[:, :, 1], in0=u16v[:],
                                        scalar1=8, scalar2=0x6400,
                                        op0=AOT.logical_shift_right,
                                        op1=AOT.bitwise_or)
                w16s.append(w16)
                w_loc.extend((c, i * OPAD) for i in range(nk))

            def x_chunk(mi, c):
                x_sb = xpool.tile([P, kgx * M_TILE], fp16, tag=f"x{c}",
                                  name=f"x{mi}_{c}")
                nc.sync.dma_start(out=x_sb[:], in_=xt_re[mi][:, c, :])
                return x_sb

            def mm(ps, x_sb, kt, ks, ms, n0, nmm):
                wc, woff = w_loc[ks]
                nc.tensor.matmul(
                    ps[:, :nmm],
                    lhsT=x_sb[:, kt * M_TILE + ms * P: kt * M_TILE + ms * P + P],
                    rhs=w16s[wc][:, woff + n0: woff + n0 + nmm],
                    start=(ks == 0), stop=(ks == KSUB - 1),
                )

            def evict(mi, out_sb, g, rs):
                # rs: the CURRENT m-tile's n2-group psum; col 352 = 1024*rowsum
                ms, n_idx = groups[g]
                n0, nmm, nev = n_slices[n_idx]
                o0 = ms * O_CORE + n0
                nc.vector.scalar_tensor_tensor(
                    out=out_sb[:, o0:o0 + nev], in0=ps_list[g][:, :nev],
                    scalar=rs[:, 352:353], in1=scale_sb[:, n0:n0 + nev],
                    op0=AOT.subtract, op1=AOT.mult)
                nc.vector.tensor_add(out=out_sb[:, o0:o0 + nev],
                                     in0=out_sb[:, o0:o0 + nev],
                                     in1=bias_sb[:, n0:n0 + nev])
                if mi < N_MT - 1:
                    ps_list[g] = ps_tile(g)  # next m-tile's tile, same bank
                nc.scalar.dma_start(out=out_re[mi * MSUB + ms][:, n0:n0 + nev],
                                    in_=out_sb[:, o0:o0 + nev])

            # --- m-tile 0: k-chunk-outer over 6 concurrent PSUM groups so
            # compute starts after the first x/w chunks land and tracks the
            # weight DMA+expansion stream.  Its x rides the finer X0 plan.
            x0_tiles = []
            k0 = 0
            for c, nk in enumerate(x0_plan):
                x_sb = xpool.tile([P, nk * M_TILE], fp16, tag=f"x0_{c}",
                                  name=f"x0_{c}", bufs=1)
                nc.sync.dma_start(
                    out=x_sb[:],
                    in_=xt[0][:, k0 * M_TILE:(k0 + nk) * M_TILE])
                x0_tiles.append(x_sb)
                k0 += nk
            # scale/bias broadcasts ride the sync ring after m-tile 0's x,
            # before the m-tile 1 prefetch (needed by first eviction ~40us).
            scale_sb = cpool.tile([P, O_CORE], f32, tag="scale_sb")
            nc.sync.dma_start(out=scale_sb[:],
                              in_=scale[None, :].to_broadcast((P, O_CORE)))
            bias_sb = cpool.tile([P, O_CORE], f32, tag="bias_sb")
            nc.sync.dma_start(out=bias_sb[:],
                              in_=bias[None, :].to_broadcast((P, O_CORE)))
            out_sb = opool.tile([P, MSUB * O_CORE], f32, tag="o", name="o0")
            k0 = 0
            for c, nk in enumerate(x0_plan):
                for g, (ms, n_idx) in enumerate(groups):
                    n0, nmm, _ = n_slices[n_idx]
                    for kt in range(nk):
                        mm(ps_list[g], x0_tiles[c], kt, k0 + kt, ms, n0, nmm)
                k0 += nk
            rs0 = [ps_list[ms * 3 + 2] for ms in range(MSUB)]
            for g, (ms, n_idx) in enumerate(groups):
                evict(0, out_sb, g, rs0[ms])

            # --- steady state: group-outer (full-K accumulation per group).
            # Within each ms-half the n2 group (whose col 352 carries the
            # rowsum term every eviction reads) runs FIRST and evicts
            # immediately (the pool keeps its bank readable until every rs
            # reader is done), so only n1's eviction trails the half's last
            # matmul (minimizes the kernel tail).
            for mi in range(1, N_MT):
                xc = [x_chunk(mi, c) for c in range(nxc)]
                out_sb = opool.tile([P, MSUB * O_CORE], f32, tag="o",
                                    name=f"o{mi}")
                for ms in range(MSUB):
                    rs = None
                    for n_idx in (2, 0, 1):
                        g = ms * 3 + n_idx
                        n0, nmm, _ = n_slices[n_idx]
                        for ks in range(KSUB):
                            mm(ps_list[g], xc[ks // kgx], ks % kgx, ks, ms,
                               n0, nmm)
                        if rs is None:
                            rs = ps_list[g]
                        evict(mi, out_sb, g, rs)
    nc.compile()
    return nc


def _shard_inputs(x2d, w, scale, bias, n_cores=N_CORES, o_core=O_CORE,
                  kgx=KGX, w_plan=W_PLAN):
    # x: [TOK, IN] f32 -> fp16, tiled [N_MT, P, KSUB*M_TILE] with free dim
    # grouped as (chunk, ktile-in-chunk, token) so each chunk is contiguous.
    xt = np.ascontiguousarray(x2d.T).astype(np.float16)       # [IN, TOK]
    xt = xt.reshape(KSUB, P, N_MT, M_TILE)                     # ks p m t
    xt = xt.transpose(2, 1, 0, 3)                              # m p ks t
    xt = np.ascontiguousarray(xt.reshape(N_MT, P, KSUB * M_TILE))
    in_maps = []
    for c in range(n_cores):
        sl = slice(c * o_core, (c + 1) * o_core)
        wtc = np.ascontiguousarray(w[sl].T).astype(np.uint8)   # [IN, o_core]
        wpad = np.zeros((KSUB, P, OPAD), dtype=np.uint8)
        wpad[:, :, :o_core] = wtc.reshape(KSUB, P, o_core)     # pad cols = 0
        by_nk = {1: [], 2: [], 4: []}
        k0 = 0
        for nk in w_plan:
            chunk = wpad[k0:k0 + nk].transpose(1, 0, 2).reshape(P, nk * OPAD)
            by_nk[nk].append(chunk)
            k0 += nk
        def stackc(lst, nk):
            if not lst:
                return np.zeros((1, P, nk * OPAD), dtype=np.uint8)
            return np.ascontiguousarray(np.stack(lst))
        in_maps.append({
            "xt": xt,
            "wt1": stackc(by_nk[1], 1),
            "wt2": stackc(by_nk[2], 2),
            "wt4": stackc(by_nk[4], 4),
            "scale": np.ascontiguousarray(scale[sl]).astype(np.float32),
            "bias": np.ascontiguousarray(bias[sl]).astype(np.float32),
        })
    return in_maps


def _ensure_ntff_hook():
    """Register the axon NTFF profiling hook if the image's antenv lacks it."""
    import sys, types
    try:
        from antenv.axon_hooks import get_axon_ntff_profile_hook  # noqa: F401
        return
    except ImportError:
        pass
    try:
        import antenv
        from trn_agent_boot.trn_boot import _ntff_profile_via_ctypes
        mod = types.ModuleType("antenv.axon_hooks")
        _hook = [_ntff_profile_via_ctypes("/opt/axon/libaxon_pjrt.so")]
        mod.set_axon_ntff_profile_hook = lambda h: _hook.__setitem__(0, h)
        mod.get_axon_ntff_profile_hook = lambda: _hook[0]
        sys.modules["antenv.axon_hooks"] = mod
        antenv.axon_hooks = mod
    except Exception as e:  # profiling is best-effort; execution still works
        print(f"NTFF hook registration failed: {e}")


def run_hw(x2d, w, scale, bias, trace=False, **build_kwargs):
    """Run sharded on 8 cores; returns (full [TOK, OUT] f32 output, exec_time_ns)."""
    if trace:
        _ensure_ntff_hook()
    nc = build_nc(**build_kwargs)
    in_maps = _shard_inputs(x2d, w, scale, bias,
                            kgx=build_kwargs.get("kgx", KGX),
                            w_plan=build_kwargs.get("w_plan", W_PLAN))
    last_err = None
    for attempt in range(3):
        try:
            res = run_bass_kernel_spmd(nc, in_maps, core_ids=list(range(N_CORES)),
                                       trace=trace)
            out = np.concatenate([res.results[c]["out"] for c in range(N_CORES)],
                                 axis=1)
            return out, res.exec_time_ns
        except Exception as e:  # transient NRT_EXEC_UNIT_UNRECOVERABLE etc.
            last_err = e
            print(f"run attempt {attempt} failed: {type(e).__name__}: {e}")
            try:
                import jax
                import jax.extend.backend as _jb
                jax.clear_caches()
                _jb.clear_backends()
            except Exception as e2:
                print(f"backend reset failed: {e2}")
            import time
            time.sleep(5)
    raise last_err


def kernel(**inputs):
    x = np.asarray(inputs["x"], dtype=np.float32)
    w = np.asarray(inputs["weight_int8"])
    scale = np.asarray(inputs["scale"], dtype=np.float32)
    bias = np.asarray(inputs["bias"], dtype=np.float32)
    out2d, _ = run_hw(x.reshape(TOK, IN), w, scale, bias, trace=False)
    return out2d.reshape(B, S, OUT)


# revision 35
# speedup vs baseline: 1.0059x; 1.0002x over previous
"""Trainium2 Bass kernel for CompressedLinear: out = x @ (w_int8 * scale).T + bias.

Sharding (Megatron column-parallel): weight/scale/bias are split along the
output dim across 8 NeuronCores, x is replicated, per-core outputs are
concatenated on the feature axis.

Identity: x @ (w*scale).T + bias == (x @ w.T) * scale + bias, so the matmul
runs on integer codes (exact in fp16) and per-channel scale/bias are applied
on PSUM eviction.  x is fp16 (~2.5e-4 rel err; budget 2e-2).

Weight transport: w ships as uint8 codes (5.6MB vs 11.3MB fp16) so the
startup burst fits under the DMA rate while m-tile 0 computes.  On-chip
expansion uses the offset-1024 trick: for c in [0,127], fp16(1024+c) has bit
pattern 0x6400|c, so two full-rate DVE tensor_scalar ops per chunk (reading
u8 pairs as u16) build fp16 weights w' = 1024+c.  The matmul then computes
x@w'.T = x@w.T + 1024*rowsum(x); a host-planted zero code per k-tile makes
column 352 of each n2 PSUM group accumulate exactly 1024*rowsum(x), and
eviction computes (psum - col352)*scale + bias via scalar_tensor_tensor.

Schedule: steady state hits the PE issue-rate roofline (6144 matmuls,
128x128 stationary x, <=512-wide moving w, 32-deep K accumulation).  Edge
optimizations: pre-tiled DRAM layouts (contiguous per-partition DMA
packets), startup split across hardware rings (x+broadcasts on sync, w on
scalar) in first-consumption order, m-tile 0 k-chunk-outer across 6
concurrent PSUM banks, and HAM warmup matmuls so the PE clock gate is at
2.4GHz when real matmuls start.
"""

import numpy as np

import concourse.bass as bass
import concourse.mybir as mybir
import concourse.tile as tile
from concourse import bacc
from concourse.bass_utils import run_bass_kernel_spmd

B, S, IN, OUT = 4, 2048, 4096, 11008
N_CORES = 8
TOK = B * S
O_CORE = OUT // N_CORES
P = 128
OPAD = O_CORE + 2     # per-ktile row: 1376 codes + zero (ones col) + pad

M_TILE = 256          # tokens per m-tile (2 PSUM-partition subtiles)
N_TILE = 512          # output columns per PSUM bank
KGX = 4               # k-tiles per x DMA chunk
N_WARM = 84           # HAM warmup matmuls issued before real data lands
X_BUFS = 2
# w chunk plan in k-tiles: small leading chunks cut time-to-first-matmul
# during the DMA ramp, 4-ktile chunks amortize the rest.
W_PLAN = (1, 1, 2, 4, 4, 4, 4, 4, 4, 4)
# m-tile 0's x chunk plan (same idea; later m-tiles use uniform KGX chunks)
X0_PLAN = (4, 4, 4, 4, 4, 4, 4, 4)

KSUB = IN // P        # 32 k-tiles
MSUB = M_TILE // P    # 2
N_MT = TOK // M_TILE  # 32 m-tiles


def build_nc(n_warm=N_WARM, x_bufs=X_BUFS, kgx=KGX, w_plan=W_PLAN,
             x0_plan=X0_PLAN):
    nxc = KSUB // kgx
    assert sum(w_plan) == KSUB and sum(x0_plan) == KSUB
    # (col offset within a ktile row, matmul width, evict width)
    n_slices = [(0, 512, 512), (512, 512, 512), (1024, 353, 352)]
    groups = [(ms, n_idx) for ms in range(MSUB) for n_idx in range(3)]
    fp16 = mybir.dt.float16
    u16 = mybir.dt.uint16
    f32 = mybir.dt.float32
    AOT = mybir.AluOpType

    nc = bacc.Bacc(None, target_bir_lowering=False)
    # x pre-tiled: [m-tile, partition, kchunk-major free] (see _shard_inputs)
    xt = nc.declare_dram_parameter("xt", [N_MT, P, KSUB * M_TILE], fp16, False)
    # w codes, chunk-contiguous per the chunk plan (per-partition contiguous
    # DMA packets: nk*1378 bytes), one param per chunk-size class.
    n1c = sum(1 for nk in w_plan if nk == 1)
    n2c = sum(1 for nk in w_plan if nk == 2)
    n4c = sum(1 for nk in w_plan if nk == 4)
    wt1 = nc.declare_dram_parameter("wt1", [max(n1c, 1), P, OPAD],
                                    mybir.dt.uint8, False)
    wt2 = nc.declare_dram_parameter("wt2", [max(n2c, 1), P, 2 * OPAD],
                                    mybir.dt.uint8, False)
    wt4 = nc.declare_dram_parameter("wt4", [max(n4c, 1), P, 4 * OPAD],
                                    mybir.dt.uint8, False)
    scale = nc.declare_dram_parameter("scale", [O_CORE], f32, False)
    bias = nc.declare_dram_parameter("bias", [O_CORE], f32, False)
    out = nc.declare_dram_parameter("out", [TOK, O_CORE], f32, True)

    xt_re = xt.rearrange("m p (c e) -> m p c e", c=nxc)   # e = kgx*M_TILE
    out_re = out.rearrange("(m p) o -> m p o", p=P)

    with tile.TileContext(nc) as tc:
        with (
            tc.tile_pool(name="const", bufs=1) as cpool,
            tc.tile_pool(name="u8p", bufs=2) as u8pool,
            tc.tile_pool(name="xp", bufs=x_bufs) as xpool,
            tc.tile_pool(name="op", bufs=2) as opool,
            tc.tile_pool(name="ps", bufs=1, space="PSUM") as pspool,
        ):
            # --- PSUM banks: 6 accumulation groups (allocated first => bank
            # aligned), warmup bank last.
            def ps_tile(g):
                return pspool.tile([P, N_TILE], f32, tag=f"ps{g}", name=f"ps{g}")

            ps_list = [ps_tile(g) for g in range(len(groups))]

            # --- HAM warmup: PE busy from the end of the framework preamble
            # so the clock gate is at 8/8 before real matmuls start.  Sized
            # to bridge until the first w chunk is expanded; results are
            # never read.
            if n_warm:
                warm_sb = cpool.tile([P, P], fp16, tag="warm_sb")
                nc.vector.memset(warm_sb[:], 0.0)
                warm_ps = pspool.tile([P, 64], f32, tag="warm_ps")
                for _ in range(n_warm):
                    nc.tensor.matmul(warm_ps[:], lhsT=warm_sb[:],
                                     rhs=warm_sb[:, :64], start=True, stop=True)

            # --- startup DMAs, first-consumption order.
            # w uint8 chunks on the scalar ring, then expansion codes ->
            # fp16(1024+c) on the vector engine: u8 pairs read as u16, two
            # strided bitwise tensor_scalar ops per chunk.
            u8s = []
            w_srcs = {1: (wt1, 0), 2: (wt2, 0), 4: (wt4, 0)}
            for c, nk in enumerate(w_plan):
                u8 = u8pool.tile([P, nk * OPAD], mybir.dt.uint8, tag=f"u8_{nk}",
                                 name=f"u8_{c}", bufs=(2 if nk < 4 else 4))
                src, idx = w_srcs[nk]
                w_srcs[nk] = (src, idx + 1)
                nc.scalar.dma_start(out=u8[:], in_=src[idx])
                u8s.append(u8)
            w16s = []       # per chunk
            w_loc = []      # per ktile: (chunk idx, offset of ktile row)
            for c, nk in enumerate(w_plan):
                w16 = cpool.tile([P, nk * OPAD], fp16, tag=f"w{c}",
                                 name=f"w16_{c}")
                u16v = u8s[c].bitcast(u16)
                wv = w16.bitcast(u16).rearrange("p (n two) -> p n two", two=2)
                nc.vector.tensor_scalar(out=wv[:, :, 0], in0=u16v[:],
                                        scalar1=0x7F, scalar2=0x6400,
                                        op0=AOT.bitwise_and, op1=AOT.bitwise_or)
                nc.vector.tensor_scalar(out=wv[:, :, 1], in0=u16v[:],
                                        scalar1=8, scalar2=0x6400,
                                        op0=AOT.logical_shift_right,
                                        op1=AOT.bitwise_or)
                w16s.append(w16)
                w_loc.extend((c, i * OPAD) for i in range(nk))

            def x_chunk(mi, c):
                x_sb = xpool.tile([P, kgx * M_TILE], fp16, tag=f"x{c}",
                                  name=f"x{mi}_{c}")
                nc.sync.dma_start(out=x_sb[:], in_=xt_re[mi][:, c, :])
                return x_sb

            def mm(ps, x_sb, kt, ks, ms, n0, nmm):
                wc, woff = w_loc[ks]
                nc.tensor.matmul(
                    ps[:, :nmm],
                    lhsT=x_sb[:, kt * M_TILE + ms * P: kt * M_TILE + ms * P + P],
                    rhs=w16s[wc][:, woff + n0: woff + n0 + nmm],
                    start=(ks == 0), stop=(ks == KSUB - 1),
                )

            def evict(mi, out_sb, g, rs, halves=1):
                # rs: the CURRENT m-tile's n2-group psum; col 352 = 1024*rowsum
                ms, n_idx = groups[g]
                n0, nmm, nev = n_slices[n_idx]
                o0 = ms * O_CORE + n0
                hw = (nev + halves - 1) // halves
                for h0 in range(0, nev, hw):
                    hn = min(hw, nev - h0)
                    nc.vector.scalar_tensor_tensor(
                        out=out_sb[:, o0 + h0:o0 + h0 + hn],
                        in0=ps_list[g][:, h0:h0 + hn],
                        scalar=rs[:, 352:353],
                        in1=scale_sb[:, n0 + h0:n0 + h0 + hn],
                        op0=AOT.subtract, op1=AOT.mult)
                    nc.vector.tensor_add(out=out_sb[:, o0 + h0:o0 + h0 + hn],
                                         in0=out_sb[:, o0 + h0:o0 + h0 + hn],
                                         in1=bias_sb[:, n0 + h0:n0 + h0 + hn])
                    nc.scalar.dma_start(
                        out=out_re[mi * MSUB + ms][:, n0 + h0:n0 + h0 + hn],
                        in_=out_sb[:, o0 + h0:o0 + h0 + hn])
                if mi < N_MT - 1:
                    ps_list[g] = ps_tile(g)  # next m-tile's tile, same bank

            # --- m-tile 0: k-chunk-outer over 6 concurrent PSUM groups so
            # compute starts after the first x/w chunks land and tracks the
            # weight DMA+expansion stream.  Its x rides the finer X0 plan.
            x0_tiles = []
            k0 = 0
            for c, nk in enumerate(x0_plan):
                x_sb = xpool.tile([P, nk * M_TILE], fp16, tag=f"x0_{c}",
                                  name=f"x0_{c}", bufs=1)
                nc.sync.dma_start(
                    out=x_sb[:],
                    in_=xt[0][:, k0 * M_TILE:(k0 + nk) * M_TILE])
                x0_tiles.append(x_sb)
                k0 += nk
            # scale/bias broadcasts ride the sync ring after m-tile 0's x,
            # before the m-tile 1 prefetch (needed by first eviction ~40us).
            scale_sb = cpool.tile([P, O_CORE], f32, tag="scale_sb")
            nc.sync.dma_start(out=scale_sb[:],
                              in_=scale[None, :].to_broadcast((P, O_CORE)))
            bias_sb = cpool.tile([P, O_CORE], f32, tag="bias_sb")
            nc.sync.dma_start(out=bias_sb[:],
                              in_=bias[None, :].to_broadcast((P, O_CORE)))
            out_sb = opool.tile([P, MSUB * O_CORE], f32, tag="o", name="o0")
            k0 = 0
            for c, nk in enumerate(x0_plan):
                for g, (ms, n_idx) in enumerate(groups):
                    n0, nmm, _ = n_slices[n_idx]
                    for kt in range(nk):
                        mm(ps_list[g], x0_tiles[c], kt, k0 + kt, ms, n0, nmm)
                k0 += nk
            rs0 = [ps_list[ms * 3 + 2] for ms in range(MSUB)]
            for g, (ms, n_idx) in enumerate(groups):
                evict(0, out_sb, g, rs0[ms])

            # --- steady state: group-outer (full-K accumulation per group).
            # Within each ms-half the n2 group (whose col 352 carries the
            # rowsum term every eviction reads) runs FIRST and evicts
            # immediately (the pool keeps its bank readable until every rs
            # reader is done), so only n1's eviction trails the half's last
            # matmul (minimizes the kernel tail).
            for mi in range(1, N_MT):
                xc = [x_chunk(mi, c) for c in range(nxc)]
                out_sb = opool.tile([P, MSUB * O_CORE], f32, tag="o",
                                    name=f"o{mi}")
                for ms in range(MSUB):
                    rs = None
                    for n_idx in (2, 0, 1):
                        g = ms * 3 + n_idx
                        n0, nmm, _ = n_slices[n_idx]
                        for ks in range(KSUB):
                            mm(ps_list[g], xc[ks // kgx], ks % kgx, ks, ms,
                               n0, nmm)
                        if rs is None:
                            rs = ps_list[g]
                        last = (mi == N_MT - 1 and ms == MSUB - 1
                                and n_idx == 1)
                        evict(mi, out_sb, g, rs, halves=2 if last else 1)
    nc.compile()
    return nc


def _shard_inputs(x2d, w, scale, bias, n_cores=N_CORES, o_core=O_CORE,
                  kgx=KGX, w_plan=W_PLAN):
    # x: [TOK, IN] f32 -> fp16, tiled [N_MT, P, KSUB*M_TILE] with free dim
    # grouped as (chunk, ktile-in-chunk, token) so each chunk is contiguous.
    xt = np.ascontiguousarray(x2d.T).astype(np.float16)       # [IN, TOK]
    xt = xt.reshape(KSUB, P, N_MT, M_TILE)                     # ks p m t
    xt = xt.transpose(2, 1, 0, 3)                              # m p ks t
    xt = np.ascontiguousarray(xt.reshape(N_MT, P, KSUB * M_TILE))
    in_maps = []
    for c in range(n_cores):
        sl = slice(c * o_core, (c + 1) * o_core)
        wtc = np.ascontiguousarray(w[sl].T).astype(np.uint8)   # [IN, o_core]
        wpad = np.zeros((KSUB, P, OPAD), dtype=np.uint8)
        wpad[:, :, :o_core] = wtc.reshape(KSUB, P, o_core)     # pad cols = 0
        by_nk = {1: [], 2: [], 4: []}
        k0 = 0
        for nk in w_plan:
            chunk = wpad[k0:k0 + nk].transpose(1, 0, 2).reshape(P, nk * OPAD)
            by_nk[nk].append(chunk)
            k0 += nk
        def stackc(lst, nk):
            if not lst:
                return np.zeros((1, P, nk * OPAD), dtype=np.uint8)
            return np.ascontiguousarray(np.stack(lst))
        in_maps.append({
            "xt": xt,
            "wt1": stackc(by_nk[1], 1),
            "wt2": stackc(by_nk[2], 2),
            "wt4": stackc(by_nk[4], 4),
            "scale": np.ascontiguousarray(scale[sl]).astype(np.float32),
            "bias": np.ascontiguousarray(bias[sl]).astype(np.float32),
        })
    return in_maps


def _ensure_ntff_hook():
    """Register the axon NTFF profiling hook if the image's antenv lacks it."""
    import sys, types
    try:
        from antenv.axon_hooks import get_axon_ntff_profile_hook  # noqa: F401
        return
    except ImportError:
        pass
    try:
        import antenv
        from trn_agent_boot.trn_boot import _ntff_profile_via_ctypes
        mod = types.ModuleType("antenv.axon_hooks")
        _hook = [_ntff_profile_via_ctypes("/opt/axon/libaxon_pjrt.so")]
        mod.set_axon_ntff_profile_hook = lambda h: _hook.__setitem__(0, h)
        mod.get_axon_ntff_profile_hook = lambda: _hook[0]
        sys.modules["antenv.axon_hooks"] = mod
        antenv.axon_hooks = mod
    except Exception as e:  # profiling is best-effort; execution still works
        print(f"NTFF hook registration failed: {e}")


def run_hw(x2d, w, scale, bias, trace=False, **build_kwargs):
    """Run sharded on 8 cores; returns (full [TOK, OUT] f32 output, exec_time_ns)."""
    if trace:
        _ensure_ntff_hook()
    nc = build_nc(**build_kwargs)
    in_maps = _shard_inputs(x2d, w, scale, bias,
                            kgx=build_kwargs.get("kgx", KGX),
                            w_plan=build_kwargs.get("w_plan", W_PLAN))
    last_err = None
    for attempt in range(3):
        try:
            res = run_bass_kernel_spmd(nc, in_maps, core_ids=list(range(N_CORES)),
                                       trace=trace)
            out = np.concatenate([res.results[c]["out"] for c in range(N_CORES)],
                                 axis=1)
            return out, res.exec_time_ns
        except Exception as e:  # transient NRT_EXEC_UNIT_UNRECOVERABLE etc.
            last_err = e
            print(f"run attempt {attempt} failed: {type(e).__name__}: {e}")
            try:
                import jax
                import jax.extend.backend as _jb
                jax.clear_caches()
                _jb.clear_backends()
            except Exception as e2:
                print(f"backend reset failed: {e2}")
            import time
            time.sleep(5)
    raise last_err


def kernel(**inputs):
    x = np.asarray(inputs["x"], dtype=np.float32)
    w = np.asarray(inputs["weight_int8"])
    scale = np.asarray(inputs["scale"], dtype=np.float32)
    bias = np.asarray(inputs["bias"], dtype=np.float32)
    out2d, _ = run_hw(x.reshape(TOK, IN), w, scale, bias, trace=False)
    return out2d.reshape(B, S, OUT)


# revision 36
# speedup vs baseline: 1.0066x; 1.0007x over previous
"""Trainium2 Bass kernel for CompressedLinear: out = x @ (w_int8 * scale).T + bias.

Sharding (Megatron column-parallel): weight/scale/bias are split along the
output dim across 8 NeuronCores, x is replicated, per-core outputs are
concatenated on the feature axis.

Identity: x @ (w*scale).T + bias == (x @ w.T) * scale + bias, so the matmul
runs on integer codes (exact in fp16) and per-channel scale/bias are applied
on PSUM eviction.  x is fp16 (~2.5e-4 rel err; budget 2e-2).

Weight transport: w ships as uint8 codes (5.6MB vs 11.3MB fp16) so the
startup burst fits under the DMA rate while m-tile 0 computes.  On-chip
expansion uses the offset-1024 trick: for c in [0,127], fp16(1024+c) has bit
pattern 0x6400|c, so two full-rate DVE tensor_scalar ops per chunk (reading
u8 pairs as u16) build fp16 weights w' = 1024+c.  The matmul then computes
x@w'.T = x@w.T + 1024*rowsum(x); a host-planted zero code per k-tile makes
column 352 of each n2 PSUM group accumulate exactly 1024*rowsum(x), and
eviction computes (psum - col352)*scale + bias via scalar_tensor_tensor.

Schedule: steady state hits the PE issue-rate roofline (6144 matmuls,
128x128 stationary x, <=512-wide moving w, 32-deep K accumulation).  Edge
optimizations: pre-tiled DRAM layouts (contiguous per-partition DMA
packets), startup split across hardware rings (x+broadcasts on sync, w on
scalar) in first-consumption order, m-tile 0 k-chunk-outer across 6
concurrent PSUM banks, and HAM warmup matmuls so the PE clock gate is at
2.4GHz when real matmuls start.
"""

import numpy as np

import concourse.bass as bass
import concourse.mybir as mybir
import concourse.tile as tile
from concourse import bacc
from concourse.bass_utils import run_bass_kernel_spmd

B, S, IN, OUT = 4, 2048, 4096, 11008
N_CORES = 8
TOK = B * S
O_CORE = OUT // N_CORES
P = 128
OPAD = O_CORE + 2     # per-ktile row: 1376 codes + zero (ones col) + pad

M_TILE = 256          # tokens per m-tile (2 PSUM-partition subtiles)
N_TILE = 512          # output columns per PSUM bank
KGX = 4               # k-tiles per x DMA chunk
N_WARM = 72           # HAM warmup matmuls issued before real data lands
X_BUFS = 2
# w chunk plan in k-tiles: small leading chunks cut time-to-first-matmul
# during the DMA ramp, 4-ktile chunks amortize the rest.
W_PLAN = (2, 2, 4, 4, 4, 4, 4, 4, 4)
# m-tile 0's x chunk plan (same idea; later m-tiles use uniform KGX chunks)
X0_PLAN = (4, 4, 4, 4, 4, 4, 4, 4)

KSUB = IN // P        # 32 k-tiles
MSUB = M_TILE // P    # 2
N_MT = TOK // M_TILE  # 32 m-tiles


def build_nc(n_warm=N_WARM, x_bufs=X_BUFS, kgx=KGX, w_plan=W_PLAN,
             x0_plan=X0_PLAN):
    nxc = KSUB // kgx
    assert sum(w_plan) == KSUB and sum(x0_plan) == KSUB
    # (col offset within a ktile row, matmul width, evict width)
    n_slices = [(0, 512, 512), (512, 512, 512), (1024, 353, 352)]
    groups = [(ms, n_idx) for ms in range(MSUB) for n_idx in range(3)]
    fp16 = mybir.dt.float16
    u16 = mybir.dt.uint16
    f32 = mybir.dt.float32
    AOT = mybir.AluOpType

    nc = bacc.Bacc(None, target_bir_lowering=False)
    # x pre-tiled: [m-tile, partition, kchunk-major free] (see _shard_inputs)
    xt = nc.declare_dram_parameter("xt", [N_MT, P, KSUB * M_TILE], fp16, False)
    # w codes, chunk-contiguous per the chunk plan (per-partition contiguous
    # DMA packets: nk*1378 bytes), one param per chunk-size class.
    n1c = sum(1 for nk in w_plan if nk == 1)
    n2c = sum(1 for nk in w_plan if nk == 2)
    n4c = sum(1 for nk in w_plan if nk == 4)
    wt1 = nc.declare_dram_parameter("wt1", [max(n1c, 1), P, OPAD],
                                    mybir.dt.uint8, False)
    wt2 = nc.declare_dram_parameter("wt2", [max(n2c, 1), P, 2 * OPAD],
                                    mybir.dt.uint8, False)
    wt4 = nc.declare_dram_parameter("wt4", [max(n4c, 1), P, 4 * OPAD],
                                    mybir.dt.uint8, False)
    scale = nc.declare_dram_parameter("scale", [O_CORE], f32, False)
    bias = nc.declare_dram_parameter("bias", [O_CORE], f32, False)
    out = nc.declare_dram_parameter("out", [TOK, O_CORE], f32, True)

    xt_re = xt.rearrange("m p (c e) -> m p c e", c=nxc)   # e = kgx*M_TILE
    out_re = out.rearrange("(m p) o -> m p o", p=P)

    with tile.TileContext(nc) as tc:
        with (
            tc.tile_pool(name="const", bufs=1) as cpool,
            tc.tile_pool(name="u8p", bufs=2) as u8pool,
            tc.tile_pool(name="xp", bufs=x_bufs) as xpool,
            tc.tile_pool(name="op", bufs=2) as opool,
            tc.tile_pool(name="ps", bufs=1, space="PSUM") as pspool,
        ):
            # --- PSUM banks: 6 accumulation groups (allocated first => bank
            # aligned), warmup bank last.
            def ps_tile(g):
                return pspool.tile([P, N_TILE], f32, tag=f"ps{g}", name=f"ps{g}")

            ps_list = [ps_tile(g) for g in range(len(groups))]

            # --- HAM warmup: PE busy from the end of the framework preamble
            # so the clock gate is at 8/8 before real matmuls start.  Sized
            # to bridge until the first w chunk is expanded; results are
            # never read.
            if n_warm:
                warm_sb = cpool.tile([P, P], fp16, tag="warm_sb")
                nc.vector.memset(warm_sb[:], 0.0)
                warm_ps = pspool.tile([P, 64], f32, tag="warm_ps")
                for _ in range(n_warm):
                    nc.tensor.matmul(warm_ps[:], lhsT=warm_sb[:],
                                     rhs=warm_sb[:, :64], start=True, stop=True)

            # --- startup DMAs, first-consumption order.
            # w uint8 chunks on the scalar ring, then expansion codes ->
            # fp16(1024+c) on the vector engine: u8 pairs read as u16, two
            # strided bitwise tensor_scalar ops per chunk.
            u8s = []
            w_srcs = {1: (wt1, 0), 2: (wt2, 0), 4: (wt4, 0)}
            for c, nk in enumerate(w_plan):
                u8 = u8pool.tile([P, nk * OPAD], mybir.dt.uint8, tag=f"u8_{nk}",
                                 name=f"u8_{c}", bufs=(2 if nk < 4 else 4))
                src, idx = w_srcs[nk]
                w_srcs[nk] = (src, idx + 1)
                nc.scalar.dma_start(out=u8[:], in_=src[idx])
                u8s.append(u8)
            w16s = []       # per chunk
            w_loc = []      # per ktile: (chunk idx, offset of ktile row)
            HOP = OPAD // 2
            for c, nk in enumerate(w_plan):
                w16 = cpool.tile([P, nk * OPAD], fp16, tag=f"w{c}",
                                 name=f"w16_{c}")
                u16v = u8s[c].bitcast(u16)
                wv = w16.bitcast(u16).rearrange("p (n two) -> p n two", two=2)
                for i in range(nk):  # per-ktile ops pipeline the MM stream
                    sl = slice(i * HOP, (i + 1) * HOP)
                    nc.vector.tensor_scalar(out=wv[:, sl, 0], in0=u16v[:, sl],
                                            scalar1=0x7F, scalar2=0x6400,
                                            op0=AOT.bitwise_and,
                                            op1=AOT.bitwise_or)
                    nc.vector.tensor_scalar(out=wv[:, sl, 1], in0=u16v[:, sl],
                                            scalar1=8, scalar2=0x6400,
                                            op0=AOT.logical_shift_right,
                                            op1=AOT.bitwise_or)
                w16s.append(w16)
                w_loc.extend((c, i * OPAD) for i in range(nk))

            def x_chunk(mi, c):
                x_sb = xpool.tile([P, kgx * M_TILE], fp16, tag=f"x{c}",
                                  name=f"x{mi}_{c}")
                nc.sync.dma_start(out=x_sb[:], in_=xt_re[mi][:, c, :])
                return x_sb

            def mm(ps, x_sb, kt, ks, ms, n0, nmm):
                wc, woff = w_loc[ks]
                nc.tensor.matmul(
                    ps[:, :nmm],
                    lhsT=x_sb[:, kt * M_TILE + ms * P: kt * M_TILE + ms * P + P],
                    rhs=w16s[wc][:, woff + n0: woff + n0 + nmm],
                    start=(ks == 0), stop=(ks == KSUB - 1),
                )

            def evict(mi, out_sb, g, rs, halves=1):
                # rs: the CURRENT m-tile's n2-group psum; col 352 = 1024*rowsum
                ms, n_idx = groups[g]
                n0, nmm, nev = n_slices[n_idx]
                o0 = ms * O_CORE + n0
                hw = (nev + halves - 1) // halves
                for h0 in range(0, nev, hw):
                    hn = min(hw, nev - h0)
                    nc.vector.scalar_tensor_tensor(
                        out=out_sb[:, o0 + h0:o0 + h0 + hn],
                        in0=ps_list[g][:, h0:h0 + hn],
                        scalar=rs[:, 352:353],
                        in1=scale_sb[:, n0 + h0:n0 + h0 + hn],
                        op0=AOT.subtract, op1=AOT.mult)
                    nc.vector.tensor_add(out=out_sb[:, o0 + h0:o0 + h0 + hn],
                                         in0=out_sb[:, o0 + h0:o0 + h0 + hn],
                                         in1=bias_sb[:, n0 + h0:n0 + h0 + hn])
                    nc.scalar.dma_start(
                        out=out_re[mi * MSUB + ms][:, n0 + h0:n0 + h0 + hn],
                        in_=out_sb[:, o0 + h0:o0 + h0 + hn])
                if mi < N_MT - 1:
                    ps_list[g] = ps_tile(g)  # next m-tile's tile, same bank

            # --- m-tile 0: k-chunk-outer over 6 concurrent PSUM groups so
            # compute starts after the first x/w chunks land and tracks the
            # weight DMA+expansion stream.  Its x rides the finer X0 plan.
            x0_tiles = []
            k0 = 0
            for c, nk in enumerate(x0_plan):
                x_sb = xpool.tile([P, nk * M_TILE], fp16, tag=f"x0_{c}",
                                  name=f"x0_{c}", bufs=1)
                nc.sync.dma_start(
                    out=x_sb[:],
                    in_=xt[0][:, k0 * M_TILE:(k0 + nk) * M_TILE])
                x0_tiles.append(x_sb)
                k0 += nk
            # scale/bias broadcasts ride the sync ring after m-tile 0's x,
            # before the m-tile 1 prefetch (needed by first eviction ~40us).
            scale_sb = cpool.tile([P, O_CORE], f32, tag="scale_sb")
            nc.sync.dma_start(out=scale_sb[:],
                              in_=scale[None, :].to_broadcast((P, O_CORE)))
            bias_sb = cpool.tile([P, O_CORE], f32, tag="bias_sb")
            nc.sync.dma_start(out=bias_sb[:],
                              in_=bias[None, :].to_broadcast((P, O_CORE)))
            out_sb = opool.tile([P, MSUB * O_CORE], f32, tag="o", name="o0")
            k0 = 0
            for c, nk in enumerate(x0_plan):
                for g, (ms, n_idx) in enumerate(groups):
                    n0, nmm, _ = n_slices[n_idx]
                    for kt in range(nk):
                        mm(ps_list[g], x0_tiles[c], kt, k0 + kt, ms, n0, nmm)
                k0 += nk
            rs0 = [ps_list[ms * 3 + 2] for ms in range(MSUB)]
            for g, (ms, n_idx) in enumerate(groups):
                evict(0, out_sb, g, rs0[ms])

            # --- steady state: group-outer (full-K accumulation per group).
            # Within each ms-half the n2 group (whose col 352 carries the
            # rowsum term every eviction reads) runs FIRST and evicts
            # immediately (the pool keeps its bank readable until every rs
            # reader is done), so only n1's eviction trails the half's last
            # matmul (minimizes the kernel tail).
            for mi in range(1, N_MT):
                xc = [x_chunk(mi, c) for c in range(nxc)]
                out_sb = opool.tile([P, MSUB * O_CORE], f32, tag="o",
                                    name=f"o{mi}")
                for ms in range(MSUB):
                    rs = None
                    for n_idx in (2, 0, 1):
                        g = ms * 3 + n_idx
                        n0, nmm, _ = n_slices[n_idx]
                        for ks in range(KSUB):
                            mm(ps_list[g], xc[ks // kgx], ks % kgx, ks, ms,
                               n0, nmm)
                        if rs is None:
                            rs = ps_list[g]
                        last = (mi == N_MT - 1 and ms == MSUB - 1
                                and n_idx == 1)
                        evict(mi, out_sb, g, rs, halves=2 if last else 1)
    nc.compile()
    return nc


def _shard_inputs(x2d, w, scale, bias, n_cores=N_CORES, o_core=O_CORE,
                  kgx=KGX, w_plan=W_PLAN):
    # x: [TOK, IN] f32 -> fp16, tiled [N_MT, P, KSUB*M_TILE] with free dim
    # grouped as (chunk, ktile-in-chunk, token) so each chunk is contiguous.
    xt = np.ascontiguousarray(x2d.T).astype(np.float16)       # [IN, TOK]
    xt = xt.reshape(KSUB, P, N_MT, M_TILE)                     # ks p m t
    xt = xt.transpose(2, 1, 0, 3)                              # m p ks t
    xt = np.ascontiguousarray(xt.reshape(N_MT, P, KSUB * M_TILE))
    in_maps = []
    for c in range(n_cores):
        sl = slice(c * o_core, (c + 1) * o_core)
        wtc = np.ascontiguousarray(w[sl].T).astype(np.uint8)   # [IN, o_core]
        wpad = np.zeros((KSUB, P, OPAD), dtype=np.uint8)
        wpad[:, :, :o_core] = wtc.reshape(KSUB, P, o_core)     # pad cols = 0
        by_nk = {1: [], 2: [], 4: []}
        k0 = 0
        for nk in w_plan:
            chunk = wpad[k0:k0 + nk].transpose(1, 0, 2).reshape(P, nk * OPAD)
            by_nk[nk].append(chunk)
            k0 += nk
        def stackc(lst, nk):
            if not lst:
                return np.zeros((1, P, nk * OPAD), dtype=np.uint8)
            return np.ascontiguousarray(np.stack(lst))
        in_maps.append({
            "xt": xt,
            "wt1": stackc(by_nk[1], 1),
            "wt2": stackc(by_nk[2], 2),
            "wt4": stackc(by_nk[4], 4),
            "scale": np.ascontiguousarray(scale[sl]).astype(np.float32),
            "bias": np.ascontiguousarray(bias[sl]).astype(np.float32),
        })
    return in_maps


def _ensure_ntff_hook():
    """Register the axon NTFF profiling hook if the image's antenv lacks it."""
    import sys, types
    try:
        from antenv.axon_hooks import get_axon_ntff_profile_hook  # noqa: F401
        return
    except ImportError:
        pass
    try:
        import antenv
        from trn_agent_boot.trn_boot import _ntff_profile_via_ctypes
        mod = types.ModuleType("antenv.axon_hooks")
        _hook = [_ntff_profile_via_ctypes("/opt/axon/libaxon_pjrt.so")]
        mod.set_axon_ntff_profile_hook = lambda h: _hook.__setitem__(0, h)
        mod.get_axon_ntff_profile_hook = lambda: _hook[0]
        sys.modules["antenv.axon_hooks"] = mod
        antenv.axon_hooks = mod
    except Exception as e:  # profiling is best-effort; execution still works
        print(f"NTFF hook registration failed: {e}")


def run_hw(x2d, w, scale, bias, trace=False, **build_kwargs):
    """Run sharded on 8 cores; returns (full [TOK, OUT] f32 output, exec_time_ns)."""
    if trace:
        _ensure_ntff_hook()
    nc = build_nc(**build_kwargs)
    in_maps = _shard_inputs(x2d, w, scale, bias,
                            kgx=build_kwargs.get("kgx", KGX),
                            w_plan=build_kwargs.get("w_plan", W_PLAN))
    last_err = None
    for attempt in range(3):
        try:
            res = run_bass_kernel_spmd(nc, in_maps, core_ids=list(range(N_CORES)),
                                       trace=trace)
            out = np.concatenate([res.results[c]["out"] for c in range(N_CORES)],
                                 axis=1)
            return out, res.exec_time_ns
        except Exception as e:  # transient NRT_EXEC_UNIT_UNRECOVERABLE etc.
            last_err = e
            print(f"run attempt {attempt} failed: {type(e).__name__}: {e}")
            try:
                import jax
                import jax.extend.backend as _jb
                jax.clear_caches()
                _jb.clear_backends()
            except Exception as e2:
                print(f"backend reset failed: {e2}")
            import time
            time.sleep(5)
    raise last_err


def kernel(**inputs):
    x = np.asarray(inputs["x"], dtype=np.float32)
    w = np.asarray(inputs["weight_int8"])
    scale = np.asarray(inputs["scale"], dtype=np.float32)
    bias = np.asarray(inputs["bias"], dtype=np.float32)
    out2d, _ = run_hw(x.reshape(TOK, IN), w, scale, bias, trace=False)
    return out2d.reshape(B, S, OUT)
